# revision 5
# baseline (speedup 1.0000x reference)
"""MLA decode attention (nn_AutoRegMLAttention) on 8 Trainium2 NeuronCores.

Strategy (self-contained, hardcoded for B=4, T=4096, H=4096, CQ=1536,
CKV=512, NH=32, HD=128, RD=64):

- Absorbed MLA decode: scores use qa = qC @ W_UK^T against the compressed
  cKV cache directly; values aggregate in compressed space and are
  decompressed once per head at the end. RoPE at position 0 is identity.
- Phase A is tensor-parallel over heads (4 heads/core) with W_DQ / W_DKV
  column-sharded (small AllGathers stitch cQ^T / cKV_t^T back together).
- Phase B is sequence-parallel: each core scores its 512-token slice of
  the cache for ALL 32 heads (plus the new-token column, data-masked to
  -1e9 on all but the last core), accumulates unnormalized softmax
  numerator/denominator, and a head-major ReduceScatter hands each core
  its 4 heads' context back.
- Phase C/D decompress with W_UV (1/denom folded in) and project with
  W_O; per-core partial outputs are summed on the host.
"""
import numpy as np

import concourse.bass as bass
import concourse.tile as tile
from concourse import mybir
from concourse.bass_utils import run_bass_kernel_spmd

B, T, H, CQ, CKV, NH, HD, RD = 4, 4096, 4096, 1536, 512, 32, 128, 64
NC = 8
NH_L, T_L, CQ_L, CKV_L = NH // NC, T // NC, CQ // NC, CKV // NC
SCALE = float(1.0 / np.sqrt(np.float32(HD + RD)))
F32 = mybir.dt.float32
CORE_IDS = list(range(NC))

AG2_QA, AG2_QR, AG2_KRT = 8192, 1024, 1024  # flat sizes of AG2 payload parts
AG2_TOT = AG2_QA + AG2_QR + AG2_KRT


def _split_multi_waits(nc, max_waits: int = 1):
    """This walrus build encodes at most one sync-wait per instruction; the
    Tile kernel-tail drain (and occasionally scheduled instructions) carry
    more. Hoist excess waits onto no-ops inserted just before, same engine."""
    for func in nc.m.functions:
        for block in func.blocks:
            new_list, changed = [], False
            for inst in block.instructions:
                si = inst.sync_info
                waits = list(si.on_wait) if si is not None else []
                if len(waits) > max_waits:
                    changed = True
                    for j, w in enumerate(waits[:-max_waits]):
                        new_list.append(mybir.InstNoOp(
                            name=f"{inst.name}-wsplit{j}",
                            sync_info=mybir.SyncInfo(on_wait=[w], on_update=[]),
                            bass_nofuse=True,
                            engine=inst.engine,
                        ))
                    inst.sync_info = mybir.SyncInfo(
                        on_wait=waits[-max_waits:], on_update=list(si.on_update))
                new_list.append(inst)
            if changed:
                block.instructions = new_list


def build():
    nc = bass.Bass()
    AF = mybir.ActivationFunctionType

    # ---------------- I/O ----------------
    hiddenT = nc.dram_tensor("hiddenT", [H, B], F32, kind="ExternalInput")
    wdq = nc.dram_tensor("wdq", [H, CQ_L], F32, kind="ExternalInput")
    wdkv = nc.dram_tensor("wdkv", [H, CKV_L], F32, kind="ExternalInput")
    wuqc = nc.dram_tensor("wuqc", [CQ, NH_L * HD], F32, kind="ExternalInput")
    wuqr = nc.dram_tensor("wuqr", [CQ, NH_L * RD], F32, kind="ExternalInput")
    wkr = nc.dram_tensor("wkr", [H, NH_L * RD], F32, kind="ExternalInput")
    wukt = nc.dram_tensor("wukt", [HD, NH_L, CKV], F32, kind="ExternalInput")
    wuv = nc.dram_tensor("wuv", [CKV, NH_L, HD], F32, kind="ExternalInput")
    wo = nc.dram_tensor("wo", [NH_L * HD, H], F32, kind="ExternalInput")
    ckvt_sl = nc.dram_tensor("ckvt_sl", [B, CKV, T_L], F32, kind="ExternalInput")
    ckv_sl = nc.dram_tensor("ckv_sl", [B, T_L, CKV], F32, kind="ExternalInput")
    krt_sl = nc.dram_tensor("krt_sl", [B, NH * RD, T_L], F32, kind="ExternalInput")
    mask_sl = nc.dram_tensor("mask_sl", [B, NH, T_L + 1], F32, kind="ExternalInput")
    colmask = nc.dram_tensor("colmask", [NH, T_L + 1], F32, kind="ExternalInput")
    ident128 = nc.dram_tensor("ident128", [128, 128], F32, kind="ExternalInput")
    ident32 = nc.dram_tensor("ident32", [32, 32], F32, kind="ExternalInput")

    out_part = nc.dram_tensor("out_part", [B, H], F32, kind="ExternalOutput")
    ckv_t_out = nc.dram_tensor("ckv_t_out", [B, CKV], F32, kind="ExternalOutput")
    kr_t_out = nc.dram_tensor("kr_t_out", [B, NH_L * RD], F32, kind="ExternalOutput")

    # collective bounce buffers
    ag1_in = nc.dram_tensor("ag1_in", [CQ_L + CKV_L, B], F32)
    ag1_out = nc.dram_tensor("ag1_out", [NC, CQ_L + CKV_L, B], F32,
                             addr_space="Shared")
    ag2_in = nc.dram_tensor("ag2_in", [AG2_TOT], F32)
    ag2_out = nc.dram_tensor("ag2_out", [NC, AG2_TOT], F32, addr_space="Shared")
    rs_in = nc.dram_tensor("rs_in", [NH, B, T_L + 1], F32)
    rs_out = nc.dram_tensor("rs_out", [NH_L, B, T_L + 1], F32)

    with tile.TileContext(nc) as tc:
        with tc.tile_pool(name="persist", bufs=1) as pp, \
             tc.tile_pool(name="consts", bufs=1) as cp:
            id128 = cp.tile([128, 128], F32)
            nc.sync.dma_start(id128[:], ident128[:])
            id32 = cp.tile([32, 32], F32)
            nc.sync.dma_start(id32[:], ident32[:])
            colm = cp.tile([NH, T_L + 1], F32)
            nc.sync.dma_start(colm[:], colmask[:])

            hT = pp.tile([128, H // 128, B], F32)
            nc.sync.dma_start(hT[:], hiddenT[:].rearrange("(c p) b -> p c b", p=128))

            # ---------------- phase A ----------------
            # cQT_part (CQ_L, B) and cKV_tT_part (CKV_L, B), stationary W
            cqt_sb = pp.tile([128, 2, B], F32)   # rows 0:128 / 128:192 (64 used)
            ckvtp_sb = pp.tile([CKV_L, B], F32)
            with tc.tile_pool(name="wdq_s", bufs=3) as wp, \
                 tc.tile_pool(name="psA1", bufs=1, space="PSUM") as ps:
                cq0 = ps.tile([128, B], F32)
                cq1 = ps.tile([64, B], F32)
                ckvp = ps.tile([CKV_L, B], F32)
                for kc in range(H // 128):
                    w = wp.tile([128, CQ_L], F32, tag="wdq")
                    nc.sync.dma_start(w[:], wdq[128 * kc:128 * (kc + 1), :])
                    st, sp = kc == 0, kc == H // 128 - 1
                    nc.tensor.matmul(cq0[:], w[:, 0:128], hT[:, kc, :],
                                     start=st, stop=sp)
                    nc.tensor.matmul(cq1[:], w[:, 128:192], hT[:, kc, :],
                                     start=st, stop=sp)
                    w2 = wp.tile([128, CKV_L], F32, tag="wdkv")
                    nc.sync.dma_start(w2[:], wdkv[128 * kc:128 * (kc + 1), :])
                    nc.tensor.matmul(ckvp[:], w2[:], hT[:, kc, :],
                                     start=st, stop=sp)
                nc.scalar.copy(cqt_sb[:, 0, :], cq0[:])
                nc.scalar.copy(cqt_sb[0:64, 1, :], cq1[:])
                nc.scalar.copy(ckvtp_sb[:], ckvp[:])

            nc.gpsimd.dma_start(ag1_in[0:128, :], cqt_sb[:, 0, :])
            nc.gpsimd.dma_start(ag1_in[128:192, :], cqt_sb[0:64, 1, :])
            nc.gpsimd.dma_start(ag1_in[192:256, :], ckvtp_sb[:])
            nc.gpsimd.collective_compute(
                "AllGather", mybir.AluOpType.bypass, replica_groups=[CORE_IDS],
                ins=[ag1_in[:]], outs=[ag1_out[:]])

            # gather cQT (CQ, B) -> [128, 12, B] and cKV_tT -> [128, 4vc, B]
            cqt = pp.tile([128, CQ // 128, B], F32)
            for cc in range(NC):
                j = 0
                while j < CQ_L:
                    q = CQ_L * cc + j
                    p0, c0 = q % 128, q // 128
                    seg = min(CQ_L - j, 128 - p0)
                    nc.gpsimd.dma_start(cqt[p0:p0 + seg, c0, :],
                                        ag1_out[cc, j:j + seg, :])
                    j += seg
            ckvtt = pp.tile([128, CKV // 128, B], F32)
            for cc in range(NC):
                p0, c0 = (64 * cc) % 128, (64 * cc) // 128
                nc.gpsimd.dma_start(ckvtt[p0:p0 + 64, c0, :],
                                    ag1_out[cc, CQ_L:CQ_L + 64, :])

            # qCT_l per head: [128 d, n, b]
            qct_sb = pp.tile([128, NH_L, B], F32)
            with tc.tile_pool(name="wuqc_s", bufs=3) as wp, \
                 tc.tile_pool(name="psA2", bufs=1, space="PSUM") as ps:
                pst = [ps.tile([128, B], F32, tag=f"qc{n}", name=f"psqc{n}") for n in range(NH_L)]
                for kc in range(CQ // 128):
                    w = wp.tile([128, NH_L * HD], F32, tag="wuqc")
                    nc.sync.dma_start(w[:], wuqc[128 * kc:128 * (kc + 1), :])
                    for n in range(NH_L):
                        nc.tensor.matmul(pst[n][:], w[:, 128 * n:128 * (n + 1)],
                                         cqt[:, kc, :],
                                         start=kc == 0, stop=kc == CQ // 128 - 1)
                for n in range(NH_L):
                    nc.scalar.copy(qct_sb[:, n, :], pst[n][:])

            # qRT_l and kR_tT_l: [(n r) 2x128, b]
            qrt_sb = pp.tile([128, 2, B], F32)
            krtt_sb = pp.tile([128, 2, B], F32)
            with tc.tile_pool(name="wuqr_s", bufs=3) as wp, \
                 tc.tile_pool(name="psA3", bufs=1, space="PSUM") as ps:
                pq = [ps.tile([128, B], F32, tag=f"qr{m}", name=f"psqr{m}") for m in range(2)]
                for kc in range(CQ // 128):
                    w = wp.tile([128, NH_L * RD], F32, tag="wuqr")
                    nc.sync.dma_start(w[:], wuqr[128 * kc:128 * (kc + 1), :])
                    for m in range(2):
                        nc.tensor.matmul(pq[m][:], w[:, 128 * m:128 * (m + 1)],
                                         cqt[:, kc, :],
                                         start=kc == 0, stop=kc == CQ // 128 - 1)
                for m in range(2):
                    nc.scalar.copy(qrt_sb[:, m, :], pq[m][:])
            with tc.tile_pool(name="wkr_s", bufs=3) as wp, \
                 tc.tile_pool(name="psA4", bufs=1, space="PSUM") as ps:
                pk = [ps.tile([128, B], F32, tag=f"kr{m}", name=f"pskr{m}") for m in range(2)]
                for kc in range(H // 128):
                    w = wp.tile([128, NH_L * RD], F32, tag="wkr")
                    nc.sync.dma_start(w[:], wkr[128 * kc:128 * (kc + 1), :])
                    for m in range(2):
                        nc.tensor.matmul(pk[m][:], w[:, 128 * m:128 * (m + 1)],
                                         hT[:, kc, :],
                                         start=kc == 0, stop=kc == H // 128 - 1)
                for m in range(2):
                    nc.scalar.copy(krtt_sb[:, m, :], pk[m][:])

            # kR_t host-layout output: [b, (m 128)] via PE transpose
            krt_row = pp.tile([B, 2, 128], F32)
            with tc.tile_pool(name="psA5", bufs=2, space="PSUM") as ps:
                for m in range(2):
                    pt = ps.tile([B, 128], F32, tag="ktr")
                    nc.tensor.transpose(pt[:], krtt_sb[:, m, :], id128[:])
                    nc.scalar.copy(krt_row[:, m, :], pt[:])
            nc.gpsimd.dma_start(kr_t_out[:], krt_row[:].rearrange("b m d -> b (m d)"))

            # qaT_l: [128 v, vc, n, b]
            wukt_sb = pp.tile([128, NH_L, CKV], F32)
            nc.sync.dma_start(wukt_sb[:], wukt[:])
            qat_sb = pp.tile([128, CKV // 128, NH_L, B], F32)
            with tc.tile_pool(name="psA6", bufs=2, space="PSUM") as ps:
                for n in range(NH_L):
                    for vc in range(CKV // 128):
                        pt = ps.tile([128, B], F32, tag="qa")
                        nc.tensor.matmul(pt[:],
                                         wukt_sb[:, n, 128 * vc:128 * (vc + 1)],
                                         qct_sb[:, n, :], start=True, stop=True)
                        nc.scalar.copy(qat_sb[:, vc, n, :], pt[:])

            # AG2: pack qaT_l + qRT_l + kR_tT_l
            for vc in range(CKV // 128):
                nc.gpsimd.dma_start(
                    ag2_in[2048 * vc:2048 * (vc + 1)].rearrange(
                        "(p n b) -> p n b", p=128, n=NH_L),
                    qat_sb[:, vc, :, :])
            for m in range(2):
                nc.gpsimd.dma_start(
                    ag2_in[AG2_QA + 512 * m:AG2_QA + 512 * (m + 1)].rearrange(
                        "(p b) -> p b", p=128),
                    qrt_sb[:, m, :])
                nc.gpsimd.dma_start(
                    ag2_in[AG2_QA + AG2_QR + 512 * m:
                           AG2_QA + AG2_QR + 512 * (m + 1)].rearrange(
                        "(p b) -> p b", p=128),
                    krtt_sb[:, m, :])
            nc.gpsimd.collective_compute(
                "AllGather", mybir.AluOpType.bypass, replica_groups=[CORE_IDS],
                ins=[ag2_in[:]], outs=[ag2_out[:]])

            # unpack: qat [128, vc, n 32, b], qrt_full/krtt_full [128, c 16, b]
            qat = pp.tile([128, CKV // 128, NH, B], F32)
            qrt_full = pp.tile([128, 16, B], F32)
            krtt_full = pp.tile([128, 16, B], F32)
            for cc in range(NC):
                for vc in range(CKV // 128):
                    nc.gpsimd.dma_start(
                        qat[:, vc, NH_L * cc:NH_L * (cc + 1), :],
                        ag2_out[cc, 2048 * vc:2048 * (vc + 1)].rearrange(
                            "(p n b) -> p n b", p=128, n=NH_L))
                for m in range(2):
                    nc.gpsimd.dma_start(
                        qrt_full[:, 2 * cc + m, :],
                        ag2_out[cc, AG2_QA + 512 * m:AG2_QA + 512 * (m + 1)]
                        .rearrange("(p b) -> p b", p=128))
                    nc.gpsimd.dma_start(
                        krtt_full[:, 2 * cc + m, :],
                        ag2_out[cc, AG2_QA + AG2_QR + 512 * m:
                                AG2_QA + AG2_QR + 512 * (m + 1)]
                        .rearrange("(p b) -> p b", p=128))

            # block-diagonal qR stationaries: [128, c 16, col 32, b]
            qr_bd = pp.tile([128, 16, NH, B], F32)
            nc.vector.memset(qr_bd[:], 0.0)
            for c in range(16):
                for h2 in range(2):
                    nc.vector.tensor_copy(
                        qr_bd[64 * h2:64 * (h2 + 1), c, 2 * c + h2, :],
                        qrt_full[64 * h2:64 * (h2 + 1), c, :])

            # cKV_t rows [1, 512] per b (for ctx rank-1 term + host output)
            ckvt_row = [pp.tile([1, CKV], F32, tag=f"ckvrow{b}", name=f"ckvrow{b}") for b in range(B)]
            with tc.tile_pool(name="psA7", bufs=2, space="PSUM") as ps:
                for vc in range(CKV // 128):
                    for b in range(B):
                        pt = ps.tile([1, 128], F32, tag="ckvr")
                        nc.tensor.transpose(pt[:], ckvtt[:, vc, b:b + 1], id128[:])
                        nc.scalar.copy(ckvt_row[b][0:1, 128 * vc:128 * (vc + 1)],
                                       pt[:])
            for b in range(B):
                nc.gpsimd.dma_start(ckv_t_out[b:b + 1, :], ckvt_row[b][:])

            # ---------------- phase B ----------------
            rs_sb = pp.tile([NH, B, T_L + 1], F32)
            with tc.tile_pool(name="ckvt_s", bufs=4) as ckvtp, \
                 tc.tile_pool(name="ckv_s", bufs=4) as ckvp, \
                 tc.tile_pool(name="krt_s", bufs=6) as krtp, \
                 tc.tile_pool(name="msk_s", bufs=2) as mskp, \
                 tc.tile_pool(name="e_s", bufs=2) as ep, \
                 tc.tile_pool(name="psB", bufs=2, space="PSUM") as psb, \
                 tc.tile_pool(name="psBtr", bufs=2, space="PSUM") as pstr, \
                 tc.tile_pool(name="psBx", bufs=1, space="PSUM") as psx:
                for b in range(B):
                    ps_s = psb.tile([NH, T_L], F32, tag="ps_s")
                    ps_x = psx.tile([NH, 1], F32, tag="ps_x")
                    for vc in range(CKV // 128):
                        ckvt_t = ckvtp.tile([128, T_L], F32, tag="ckvt")
                        nc.sync.dma_start(
                            ckvt_t[:], ckvt_sl[b, 128 * vc:128 * (vc + 1), :])
                        nc.tensor.matmul(ps_s[:], qat[:, vc, :, b], ckvt_t[:],
                                         start=vc == 0, stop=False)
                    for c in range(16):
                        krt_t = krtp.tile([128, T_L], F32, tag="krt")
                        nc.sync.dma_start(
                            krt_t[:], krt_sl[b, 128 * c:128 * (c + 1), :])
                        nc.tensor.matmul(ps_s[:], qr_bd[:, c, :, b], krt_t[:],
                                         start=False, stop=c == 15)
                    # extra (new-token) column
                    for vc in range(CKV // 128):
                        nc.tensor.matmul(ps_x[:], qat[:, vc, :, b],
                                         ckvtt[:, vc, b:b + 1],
                                         start=vc == 0, stop=False)
                    for c in range(16):
                        nc.tensor.matmul(ps_x[:], qr_bd[:, c, :, b],
                                         krtt_full[:, c, b:b + 1],
                                         start=False, stop=c == 15)

                    msk = mskp.tile([NH, T_L + 1], F32, tag="msk")
                    nc.sync.dma_start(msk[:], mask_sl[b])
                    t1 = ep.tile([NH, T_L + 1], F32, tag="t1")
                    nc.vector.scalar_tensor_tensor(
                        t1[:], msk[:], -1e9, colm[:],
                        op0=mybir.AluOpType.mult, op1=mybir.AluOpType.add)
                    s_all = ep.tile([NH, T_L + 1], F32, tag="s_all")
                    nc.vector.tensor_add(s_all[:, 0:T_L], ps_s[:], t1[:, 0:T_L])
                    nc.vector.tensor_add(s_all[:, T_L:T_L + 1], ps_x[:],
                                         t1[:, T_L:T_L + 1])
                    e_sb = ep.tile([NH, T_L + 1], F32, tag="e_sb")
                    den = ep.tile([NH, 1], F32, tag="den")
                    nc.scalar.activation(e_sb[:], s_all[:], AF.Exp,
                                         scale=SCALE, accum_out=den[:])

                    # transpose e -> eT [128, tc2, 32], and extra col [1, 32]
                    eT = ep.tile([128, T_L // 128, NH], F32, tag="eT")
                    exT = ep.tile([1, NH], F32, tag="exT")
                    for tc2 in range(T_L // 128):
                        pt = pstr.tile([128, NH], F32, tag="ps_tr")
                        nc.tensor.transpose(
                            pt[:], e_sb[:, 128 * tc2:128 * (tc2 + 1)], id32[:])
                        nc.scalar.copy(eT[:, tc2, :], pt[:])
                    ptx = psx.tile([1, NH], F32, tag="ps_trx")
                    nc.tensor.transpose(ptx[:], e_sb[:, T_L:T_L + 1], id32[:])
                    nc.scalar.copy(exT[:], ptx[:])

                    # ctxU accumulation [32, 512]
                    ps_c = psb.tile([NH, CKV], F32, tag="ps_c")
                    for tc2 in range(T_L // 128):
                        ckv_t = ckvp.tile([128, CKV], F32, tag="ckv")
                        nc.sync.dma_start(
                            ckv_t[:], ckv_sl[b, 128 * tc2:128 * (tc2 + 1), :])
                        nc.tensor.matmul(ps_c[:], eT[:, tc2, :], ckv_t[:],
                                         start=tc2 == 0, stop=False)
                    nc.tensor.matmul(ps_c[:], exT[:], ckvt_row[b][:],
                                     start=False, stop=True)
                    nc.scalar.copy(rs_sb[:, b, 0:CKV], ps_c[:])
                    nc.vector.tensor_copy(rs_sb[:, b, CKV:CKV + 1], den[:])

            nc.sync.dma_start(rs_in[:], rs_sb[:])
            nc.gpsimd.collective_compute(
                "ReduceScatter", mybir.AluOpType.add, replica_groups=[CORE_IDS],
                ins=[rs_in[:]], outs=[rs_out[:]])

            # ---------------- phase C ----------------
            ctxu = pp.tile([128, CKV // 128, NH_L, B], F32)
            for vc in range(CKV // 128):
                nc.gpsimd.dma_start(
                    ctxu[:, vc, :, :],
                    rs_out[:, :, 128 * vc:128 * (vc + 1)].rearrange(
                        "n b p -> p n b"))
            den_sb = pp.tile([1, NH_L * B], F32)
            nc.gpsimd.dma_start(
                den_sb[:], rs_out[:, :, CKV:CKV + 1].rearrange("n b one -> one (n b)"))
            recip = pp.tile([1, NH_L * B], F32)
            nc.vector.reciprocal(recip[:], den_sb[:])
            ones_sb = pp.tile([1, 128], F32)
            nc.vector.memset(ones_sb[:], 1.0)
            recip_b = pp.tile([128, NH_L, B], F32)
            with tc.tile_pool(name="psC0", bufs=1, space="PSUM") as ps:
                pr = ps.tile([128, NH_L * B], F32)
                nc.tensor.matmul(pr[:], ones_sb[:], recip[:], start=True, stop=True)
                nc.scalar.copy(
                    recip_b[:].rearrange("p n b -> p (n b)"), pr[:])

            wuv_sb = pp.tile([128, CKV // 128, NH_L, HD], F32)
            nc.sync.dma_start(
                wuv_sb[:], wuv[:].rearrange("(vc p) n d -> p vc n d", p=128))
            ao_sb = pp.tile([128, NH_L, B], F32)
            with tc.tile_pool(name="psC1", bufs=2, space="PSUM") as ps:
                for n in range(NH_L):
                    pa = ps.tile([128, B], F32, tag="ao")
                    for vc in range(CKV // 128):
                        nc.tensor.matmul(pa[:], wuv_sb[:, vc, n, :],
                                         ctxu[:, vc, n, :],
                                         start=vc == 0, stop=vc == CKV // 128 - 1)
                    nc.vector.tensor_mul(ao_sb[:, n, :], pa[:], recip_b[:, n, :])

            # ---------------- phase D ----------------
            with tc.tile_pool(name="wo_s", bufs=3) as wp, \
                 tc.tile_pool(name="o_s", bufs=2) as op_, \
                 tc.tile_pool(name="psD", bufs=2, space="PSUM") as ps:
                for hc in range(H // 512):
                    po = ps.tile([B, 512], F32, tag="po")
                    for c in range(NH_L):
                        w = wp.tile([128, 512], F32, tag="wo")
                        nc.sync.dma_start(
                            w[:], wo[128 * c:128 * (c + 1),
                                     512 * hc:512 * (hc + 1)])
                        nc.tensor.matmul(po[:], ao_sb[:, c, :], w[:],
                                         start=c == 0, stop=c == NH_L - 1)
                    o_sb = op_.tile([B, 512], F32, tag="o_sb")
                    nc.scalar.copy(o_sb[:], po[:])
                    nc.sync.dma_start(out_part[:, 512 * hc:512 * (hc + 1)], o_sb[:])

    _split_multi_waits(nc)
    return nc


_nc_cache = None


def _get_nc():
    global _nc_cache
    if _nc_cache is None:
        _nc_cache = build()
    return _nc_cache


def _prep_core_inputs(i, hid, mask, cached_cKV, cached_kR,
                      W_DQ, W_DKV, W_UQ_C, W_UQ_R, W_UK_C, W_UV_C, W_KR, W_O,
                      ident128, ident32):
    sl = slice(T_L * i, T_L * (i + 1))
    hs = slice(NH_L * i, NH_L * (i + 1))
    cm = np.zeros((NH, T_L + 1), np.float32)
    if i != NC - 1:
        cm[:, T_L] = -1e9
    return {
        "hiddenT": np.ascontiguousarray(hid.T),
        "wdq": np.ascontiguousarray(W_DQ[:, CQ_L * i:CQ_L * (i + 1)]),
        "wdkv": np.ascontiguousarray(W_DKV[:, CKV_L * i:CKV_L * (i + 1)]),
        "wuqc": np.ascontiguousarray(W_UQ_C[:, hs, :].reshape(CQ, NH_L * HD)),
        "wuqr": np.ascontiguousarray(W_UQ_R[:, hs, :].reshape(CQ, NH_L * RD)),
        "wkr": np.ascontiguousarray(W_KR[:, hs, :].reshape(H, NH_L * RD)),
        "wukt": np.ascontiguousarray(W_UK_C[:, hs, :].transpose(2, 1, 0)),
        "wuv": np.ascontiguousarray(W_UV_C[:, hs, :]),
        "wo": np.ascontiguousarray(W_O[hs].reshape(NH_L * HD, H)),
        "ckvt_sl": np.ascontiguousarray(cached_cKV[:, sl, :].transpose(0, 2, 1)),
        "ckv_sl": np.ascontiguousarray(cached_cKV[:, sl, :]),
        "krt_sl": np.ascontiguousarray(
            cached_kR[:, sl].transpose(0, 2, 3, 1).reshape(B, NH * RD, T_L)),
        "mask_sl": np.ascontiguousarray(
            np.concatenate([mask[:, 0, :, sl], mask[:, 0, :, T:T + 1]], axis=-1)),
        "colmask": cm,
        "ident128": ident128,
        "ident32": ident32,
    }


def kernel(hidden_states, mask, cached_cKV, cached_kR,
           W_DQ, W_DKV, W_UQ_C, W_UQ_R, W_UK_C, W_UV_C, W_KR, W_O):
    hidden_states = np.asarray(hidden_states, np.float32)
    mask = np.asarray(mask, np.float32)
    cached_cKV = np.asarray(cached_cKV, np.float32)
    cached_kR = np.asarray(cached_kR, np.float32)
    args = [np.asarray(a, np.float32) for a in
            (W_DQ, W_DKV, W_UQ_C, W_UQ_R, W_UK_C, W_UV_C, W_KR, W_O)]
    hid = hidden_states[:, 0, :]
    id128 = np.eye(128, dtype=np.float32)
    id32 = np.eye(32, dtype=np.float32)

    in_maps = [_prep_core_inputs(i, hid, mask, cached_cKV, cached_kR,
                                 *args, id128, id32) for i in range(NC)]
    nc = _get_nc()
    res = run_bass_kernel_spmd(nc, in_maps, CORE_IDS)
    results = res.results

    output = np.sum([results[i]["out_part"] for i in range(NC)],
                    axis=0, dtype=np.float32)[:, None, :]
    cKV_t = results[0]["ckv_t_out"]                       # (B, CKV)
    kR_t = np.concatenate(
        [results[i]["kr_t_out"].reshape(B, NH_L, RD) for i in range(NC)],
        axis=1)                                           # (B, NH, RD)
    new_cached_cKV = np.concatenate([cached_cKV, cKV_t[:, None, :]], axis=1)
    new_cached_kR = np.concatenate([cached_kR, kR_t[:, None, :, :]], axis=1)
    return output, new_cached_cKV, new_cached_kR


# revision 7
# speedup vs baseline: 1.3376x; 1.3376x over previous
"""MLA decode attention (nn_AutoRegMLAttention) on 8 Trainium2 NeuronCores.

Strategy (self-contained, hardcoded for B=4, T=4096, H=4096, CQ=1536,
CKV=512, NH=32, HD=128, RD=64):

- Absorbed MLA decode: scores use qa = qC @ W_UK^T against the compressed
  cKV cache directly; values aggregate in compressed space and are
  decompressed once per head at the end. RoPE at position 0 is identity.
- Phase A is tensor-parallel over heads (4 heads/core) with W_DQ / W_DKV
  column-sharded (small AllGathers stitch cQ^T / cKV_t^T back together).
  All projections keep the big weight as the MOVING matmul operand
  (activations are the tiny stationary side); row-form outputs are
  PE-transposed into column form.
- Phase B is sequence-parallel: each core scores its 512-token slice of
  the cache for ALL 32 heads (plus the new-token column, data-masked to
  -1e9 on all but the last core), accumulates unnormalized softmax
  numerator/denominator, and a head-major ReduceScatter hands each core
  its 4 heads' context back. Large-N matmuls run as float32r (full PE
  column rate vs 1/4 for fp32; ~1e-4 relative error).
- Phase C/D decompress with W_UV (1/denom folded in) and project with
  W_O; per-core partial outputs are summed on the host.
"""
import numpy as np

import concourse.bass as bass
import concourse.tile as tile
from concourse import mybir
from concourse.bass_utils import run_bass_kernel_spmd

B, T, H, CQ, CKV, NH, HD, RD = 4, 4096, 4096, 1536, 512, 32, 128, 64
NC = 8
NH_L, T_L, CQ_L, CKV_L = NH // NC, T // NC, CQ // NC, CKV // NC
SCALE = float(1.0 / np.sqrt(np.float32(HD + RD)))
F32 = mybir.dt.float32
F32R = mybir.dt.float32r
CORE_IDS = list(range(NC))

AG2_QA, AG2_QR, AG2_KRT = 8192, 1024, 1024  # flat sizes of AG2 payload parts
AG2_TOT = AG2_QA + AG2_QR + AG2_KRT


def _split_multi_waits(nc, max_waits: int = 1):
    """This walrus build encodes at most one sync-wait per instruction; the
    Tile kernel-tail drain (and occasionally scheduled instructions) carry
    more. Hoist excess waits onto no-ops inserted just before, same engine."""
    for func in nc.m.functions:
        for block in func.blocks:
            new_list, changed = [], False
            for inst in block.instructions:
                si = inst.sync_info
                waits = list(si.on_wait) if si is not None else []
                if len(waits) > max_waits:
                    changed = True
                    for j, w in enumerate(waits[:-max_waits]):
                        new_list.append(mybir.InstNoOp(
                            name=f"{inst.name}-wsplit{j}",
                            sync_info=mybir.SyncInfo(on_wait=[w], on_update=[]),
                            bass_nofuse=True,
                            engine=inst.engine,
                        ))
                    inst.sync_info = mybir.SyncInfo(
                        on_wait=waits[-max_waits:], on_update=list(si.on_update))
                new_list.append(inst)
            if changed:
                block.instructions = new_list


def build():
    nc = bass.Bass()
    AF = mybir.ActivationFunctionType

    # ---------------- I/O ----------------
    hiddenT = nc.dram_tensor("hiddenT", [H, B], F32, kind="ExternalInput")
    wdq = nc.dram_tensor("wdq", [H, CQ_L], F32, kind="ExternalInput")
    wdkv = nc.dram_tensor("wdkv", [H, CKV_L], F32, kind="ExternalInput")
    wuqc = nc.dram_tensor("wuqc", [CQ, NH_L * HD], F32, kind="ExternalInput")
    wuqr = nc.dram_tensor("wuqr", [CQ, NH_L * RD], F32, kind="ExternalInput")
    wkr = nc.dram_tensor("wkr", [H, NH_L * RD], F32, kind="ExternalInput")
    wukt = nc.dram_tensor("wukt", [HD, NH_L, CKV], F32, kind="ExternalInput")
    wuv = nc.dram_tensor("wuv", [CKV, NH_L, HD], F32, kind="ExternalInput")
    wo = nc.dram_tensor("wo", [NH_L * HD, H], F32R, kind="ExternalInput")
    ckvt_sl = nc.dram_tensor("ckvt_sl", [B, CKV, T_L], F32R, kind="ExternalInput")
    ckv_sl = nc.dram_tensor("ckv_sl", [B, T_L, CKV], F32R, kind="ExternalInput")
    krt_sl = nc.dram_tensor("krt_sl", [B, NH * RD, T_L], F32R, kind="ExternalInput")
    mask_sl = nc.dram_tensor("mask_sl", [B, NH, T_L + 1], F32, kind="ExternalInput")
    colmask = nc.dram_tensor("colmask", [NH, T_L + 1], F32, kind="ExternalInput")
    ident128 = nc.dram_tensor("ident128", [128, 128], F32, kind="ExternalInput")

    out_part = nc.dram_tensor("out_part", [B, H], F32, kind="ExternalOutput")
    ckv_t_out = nc.dram_tensor("ckv_t_out", [B, CKV], F32, kind="ExternalOutput")
    kr_t_out = nc.dram_tensor("kr_t_out", [B, NH_L * RD], F32, kind="ExternalOutput")

    # collective bounce buffers
    ag1_in = nc.dram_tensor("ag1_in", [CQ_L + CKV_L, B], F32)
    ag1_out = nc.dram_tensor("ag1_out", [NC, CQ_L + CKV_L, B], F32,
                             addr_space="Shared")
    ag2_in = nc.dram_tensor("ag2_in", [AG2_TOT], F32)
    ag2_out = nc.dram_tensor("ag2_out", [NC, AG2_TOT], F32, addr_space="Shared")
    rs_in = nc.dram_tensor("rs_in", [NH, B, T_L + 1], F32)
    rs_out = nc.dram_tensor("rs_out", [NH_L, B, T_L + 1], F32)

    with tile.TileContext(nc) as tc:
        with tc.tile_pool(name="persist", bufs=1) as pp, \
             tc.tile_pool(name="consts", bufs=1) as cp:
            id128 = cp.tile([128, 128], F32)
            nc.scalar.dma_start(id128[:], ident128[:])
            colm = cp.tile([NH, T_L + 1], F32)
            nc.scalar.dma_start(colm[:], colmask[:])

            hT = pp.tile([128, H // 128, B], F32)
            nc.scalar.dma_start(hT[:],
                                hiddenT[:].rearrange("(c p) b -> p c b", p=128))

            # ---------------- phase A ----------------
            # row-form projections: stationary hT/cqt chunk [128,4], moving W
            cqt_sb = pp.tile([128, 2, B], F32)   # cQT_part rows 0:128 / 128:192
            ckvtp_sb = pp.tile([CKV_L, B], F32)
            with tc.tile_pool(name="wdq_s", bufs=3) as wp, \
                 tc.tile_pool(name="psA1", bufs=1, space="PSUM") as ps, \
                 tc.tile_pool(name="psA1t", bufs=2, space="PSUM") as pst:
                ps_cq = ps.tile([B, CQ_L], F32)
                ps_ckv = ps.tile([B, CKV_L], F32)
                for g in range(8):
                    w = wp.tile([128, 4, CQ_L], F32, tag="wdq")
                    nc.scalar.dma_start(
                        w[:], wdq[512 * g:512 * (g + 1), :].rearrange(
                            "(c p) q -> p c q", p=128))
                    w2 = wp.tile([128, 4, CKV_L], F32, tag="wdkv")
                    nc.scalar.dma_start(
                        w2[:], wdkv[512 * g:512 * (g + 1), :].rearrange(
                            "(c p) q -> p c q", p=128))
                    for j in range(4):
                        kc = 4 * g + j
                        st, sp = kc == 0, kc == 31
                        nc.tensor.matmul(ps_cq[:], hT[:, kc, :], w[:, j, :],
                                         start=st, stop=sp)
                        nc.tensor.matmul(ps_ckv[:], hT[:, kc, :], w2[:, j, :],
                                         start=st, stop=sp)
                cq_row = pp.tile([B, CQ_L], F32)
                nc.scalar.copy(cq_row[:], ps_cq[:])
                ckv_row_p = pp.tile([B, CKV_L], F32)
                nc.scalar.copy(ckv_row_p[:], ps_ckv[:])
                # transpose row -> column form
                t0 = pst.tile([128, B], F32, tag="trA")
                nc.tensor.transpose(t0[:], cq_row[:, 0:128], id128[0:B, 0:B])
                nc.scalar.copy(cqt_sb[:, 0, :], t0[:])
                t1 = pst.tile([64, B], F32, tag="trA2")
                nc.tensor.transpose(t1[:], cq_row[:, 128:192], id128[0:B, 0:B])
                nc.scalar.copy(cqt_sb[0:64, 1, :], t1[:])
                t2 = pst.tile([64, B], F32, tag="trA2")
                nc.tensor.transpose(t2[:], ckv_row_p[:], id128[0:B, 0:B])
                nc.scalar.copy(ckvtp_sb[:], t2[:])

            nc.gpsimd.dma_start(ag1_in[0:128, :], cqt_sb[:, 0, :])
            nc.gpsimd.dma_start(ag1_in[128:192, :], cqt_sb[0:64, 1, :])
            nc.gpsimd.dma_start(ag1_in[192:256, :], ckvtp_sb[:])
            nc.gpsimd.collective_compute(
                "AllGather", mybir.AluOpType.bypass, replica_groups=[CORE_IDS],
                ins=[ag1_in[:]], outs=[ag1_out[:]])

            # gather cQT (CQ, B) -> [128, 12, B] and cKV_tT -> [128, 4vc, B]
            cqt = pp.tile([128, CQ // 128, B], F32)
            for cc in range(NC):
                j = 0
                while j < CQ_L:
                    q = CQ_L * cc + j
                    p0, c0 = q % 128, q // 128
                    seg = min(CQ_L - j, 128 - p0)
                    nc.gpsimd.dma_start(cqt[p0:p0 + seg, c0, :],
                                        ag1_out[cc, j:j + seg, :])
                    j += seg
            ckvtt = pp.tile([128, CKV // 128, B], F32)
            for cc in range(NC):
                p0, c0 = (64 * cc) % 128, (64 * cc) // 128
                nc.gpsimd.dma_start(ckvtt[p0:p0 + 64, c0, :],
                                    ag1_out[cc, CQ_L:CQ_L + 64, :])

            # qC/qR/kR_t rows, then transpose to column form
            qct_sb = pp.tile([128, NH_L, B], F32)
            qrt_sb = pp.tile([128, 2, B], F32)
            krtt_sb = pp.tile([128, 2, B], F32)
            kr_row = pp.tile([B, NH_L * RD], F32)
            with tc.tile_pool(name="wu_s", bufs=3) as wp, \
                 tc.tile_pool(name="psA2", bufs=1, space="PSUM") as ps, \
                 tc.tile_pool(name="psA2t", bufs=2, space="PSUM") as pst:
                ps_qc = ps.tile([B, NH_L * HD], F32)
                ps_qr = ps.tile([B, NH_L * RD], F32)
                for g in range(3):
                    w = wp.tile([128, 4, NH_L * HD], F32, tag="wuqc")
                    nc.scalar.dma_start(
                        w[:], wuqc[512 * g:512 * (g + 1), :].rearrange(
                            "(c p) q -> p c q", p=128))
                    w2 = wp.tile([128, 4, NH_L * RD], F32, tag="wuqr")
                    nc.scalar.dma_start(
                        w2[:], wuqr[512 * g:512 * (g + 1), :].rearrange(
                            "(c p) q -> p c q", p=128))
                    for j in range(4):
                        kc = 4 * g + j
                        st, sp = kc == 0, kc == 11
                        nc.tensor.matmul(ps_qc[:], cqt[:, kc, :], w[:, j, :],
                                         start=st, stop=sp)
                        nc.tensor.matmul(ps_qr[:], cqt[:, kc, :], w2[:, j, :],
                                         start=st, stop=sp)
                qc_row = pp.tile([B, NH_L * HD], F32)
                nc.scalar.copy(qc_row[:], ps_qc[:])
                qr_row = pp.tile([B, NH_L * RD], F32)
                nc.scalar.copy(qr_row[:], ps_qr[:])
                for n in range(NH_L):
                    tq = pst.tile([128, B], F32, tag="trQ")
                    nc.tensor.transpose(tq[:], qc_row[:, 128 * n:128 * (n + 1)],
                                        id128[0:B, 0:B])
                    nc.scalar.copy(qct_sb[:, n, :], tq[:])
                for m in range(2):
                    tq = pst.tile([128, B], F32, tag="trQ")
                    nc.tensor.transpose(tq[:], qr_row[:, 128 * m:128 * (m + 1)],
                                        id128[0:B, 0:B])
                    nc.scalar.copy(qrt_sb[:, m, :], tq[:])

                ps_kr = ps.tile([B, NH_L * RD], F32, tag="pskr")
                for g in range(8):
                    w = wp.tile([128, 4, NH_L * RD], F32, tag="wkr")
                    nc.scalar.dma_start(
                        w[:], wkr[512 * g:512 * (g + 1), :].rearrange(
                            "(c p) q -> p c q", p=128))
                    for j in range(4):
                        kc = 4 * g + j
                        nc.tensor.matmul(ps_kr[:], hT[:, kc, :], w[:, j, :],
                                         start=kc == 0, stop=kc == 31)
                nc.scalar.copy(kr_row[:], ps_kr[:])
                for m in range(2):
                    tq = pst.tile([128, B], F32, tag="trQ")
                    nc.tensor.transpose(tq[:], kr_row[:, 128 * m:128 * (m + 1)],
                                        id128[0:B, 0:B])
                    nc.scalar.copy(krtt_sb[:, m, :], tq[:])
            nc.gpsimd.dma_start(kr_t_out[:], kr_row[:])

            # qa rows per head -> qaT [128 v, vc, n, b]
            wukt_sb = pp.tile([128, NH_L, CKV], F32)
            nc.scalar.dma_start(wukt_sb[:], wukt[:])
            qat_sb = pp.tile([128, CKV // 128, NH_L, B], F32)
            with tc.tile_pool(name="psA3", bufs=2, space="PSUM") as ps, \
                 tc.tile_pool(name="psA3t", bufs=2, space="PSUM") as pst:
                for n in range(NH_L):
                    pa = ps.tile([B, CKV], F32, tag="qa_row")
                    nc.tensor.matmul(pa[:], qct_sb[:, n, :], wukt_sb[:, n, :],
                                     start=True, stop=True)
                    qa_row = pp.tile([B, CKV], F32, name=f"qa_row_sb{n}",
                                     tag=f"qa_row_sb{n}")
                    nc.scalar.copy(qa_row[:], pa[:])
                    for vc in range(CKV // 128):
                        tq = pst.tile([128, B], F32, tag="trQa")
                        nc.tensor.transpose(
                            tq[:], qa_row[:, 128 * vc:128 * (vc + 1)],
                            id128[0:B, 0:B])
                        nc.scalar.copy(qat_sb[:, vc, n, :], tq[:])

            # AG2: pack qaT_l + qRT_l + kR_tT_l
            for vc in range(CKV // 128):
                nc.gpsimd.dma_start(
                    ag2_in[2048 * vc:2048 * (vc + 1)].rearrange(
                        "(p n b) -> p n b", p=128, n=NH_L),
                    qat_sb[:, vc, :, :])
            nc.gpsimd.dma_start(
                ag2_in[AG2_QA:AG2_QA + AG2_QR].rearrange(
                    "(m p b) -> p m b", p=128, m=2),
                qrt_sb[:])
            nc.gpsimd.dma_start(
                ag2_in[AG2_QA + AG2_QR:AG2_TOT].rearrange(
                    "(m p b) -> p m b", p=128, m=2),
                krtt_sb[:])
            nc.gpsimd.collective_compute(
                "AllGather", mybir.AluOpType.bypass, replica_groups=[CORE_IDS],
                ins=[ag2_in[:]], outs=[ag2_out[:]])

            # unpack: qat [128, vc, n 32, b], qrt_full/krtt_full [128, c 16, b]
            qat = pp.tile([128, CKV // 128, NH, B], F32)
            qrt_full = pp.tile([128, 16, B], F32)
            krtt_full = pp.tile([128, 16, B], F32)
            for cc in range(NC):
                nc.gpsimd.dma_start(
                    qat[:, :, NH_L * cc:NH_L * (cc + 1), :],
                    ag2_out[cc, 0:AG2_QA].rearrange(
                        "(vc p n b) -> p vc n b", p=128, vc=4, n=NH_L))
                nc.gpsimd.dma_start(
                    qrt_full[:, 2 * cc:2 * (cc + 1), :],
                    ag2_out[cc, AG2_QA:AG2_QA + AG2_QR].rearrange(
                        "(m p b) -> p m b", p=128, m=2))
                nc.gpsimd.dma_start(
                    krtt_full[:, 2 * cc:2 * (cc + 1), :],
                    ag2_out[cc, AG2_QA + AG2_QR:AG2_TOT].rearrange(
                        "(m p b) -> p m b", p=128, m=2))

            # block-diagonal qR stationaries: [128, c 16, col 32, b]
            qr_bd = pp.tile([128, 16, NH, B], F32)
            nc.vector.memset(qr_bd[:], 0.0)
            for c in range(16):
                for h2 in range(2):
                    nc.vector.tensor_copy(
                        qr_bd[64 * h2:64 * (h2 + 1), c, 2 * c + h2, :],
                        qrt_full[64 * h2:64 * (h2 + 1), c, :])

            # f32r casts of score/ctx stationaries and small rhs columns
            qat_r = pp.tile([128, CKV // 128, NH, B], F32R)
            nc.scalar.copy(qat_r[:], qat[:])
            qr_bd_r = pp.tile([128, 16, NH, B], F32R)
            nc.scalar.copy(qr_bd_r[:], qr_bd[:])

            # cKV_t rows [1, 512] per b (ctx rank-1 term + host output)
            ckvt_row = [pp.tile([1, CKV], F32, name=f"ckvrow{b}",
                                tag=f"ckvrow{b}") for b in range(B)]
            ckvt_row_r = [pp.tile([1, CKV], F32R, name=f"ckvrowr{b}",
                                  tag=f"ckvrowr{b}") for b in range(B)]
            with tc.tile_pool(name="psA7", bufs=2, space="PSUM") as ps:
                for vc in range(CKV // 128):
                    for b in range(B):
                        pt = ps.tile([1, 128], F32, tag="ckvr")
                        nc.tensor.transpose(pt[:], ckvtt[:, vc, b:b + 1],
                                            id128[:])
                        nc.scalar.copy(
                            ckvt_row[b][0:1, 128 * vc:128 * (vc + 1)], pt[:])
            for b in range(B):
                nc.scalar.copy(ckvt_row_r[b][:], ckvt_row[b][:])
                nc.gpsimd.dma_start(ckv_t_out[b:b + 1, :], ckvt_row[b][:])

            # ---------------- phase B ----------------
            rs_sb = pp.tile([NH, B, T_L + 1], F32)
            msk_all = pp.tile([NH, B, T_L + 1], F32)
            nc.sync.dma_start(msk_all[:], mask_sl[:].rearrange("b n t -> n b t"))
            with tc.tile_pool(name="ckvt_s", bufs=2) as ckvtp, \
                 tc.tile_pool(name="ckv_s", bufs=2) as ckvp, \
                 tc.tile_pool(name="krt_s", bufs=3) as krtp, \
                 tc.tile_pool(name="e_s", bufs=2) as ep, \
                 tc.tile_pool(name="psB", bufs=2, space="PSUM") as psb, \
                 tc.tile_pool(name="psBtr", bufs=2, space="PSUM") as pstr, \
                 tc.tile_pool(name="psBx", bufs=1, space="PSUM") as psx:
                for b in range(B):
                    ps_s = psb.tile([NH, T_L], F32, tag="ps_s")
                    ps_x = psx.tile([NH, 1], F32, tag="ps_x")
                    ckvt_t = ckvtp.tile([128, CKV // 128, T_L], F32R, tag="ckvt")
                    nc.sync.dma_start(
                        ckvt_t[:],
                        ckvt_sl[b].rearrange("(vc p) t -> p vc t", p=128))
                    for vc in range(CKV // 128):
                        nc.tensor.matmul(ps_s[:], qat_r[:, vc, :, b],
                                         ckvt_t[:, vc, :],
                                         start=vc == 0, stop=False)
                    for cg in range(2):
                        krt_t = krtp.tile([128, 8, T_L], F32R, tag="krt")
                        nc.sync.dma_start(
                            krt_t[:],
                            krt_sl[b, 1024 * cg:1024 * (cg + 1), :].rearrange(
                                "(c p) t -> p c t", p=128))
                        for cj in range(8):
                            c = 8 * cg + cj
                            nc.tensor.matmul(ps_s[:], qr_bd_r[:, c, :, b],
                                             krt_t[:, cj, :],
                                             start=False, stop=c == 15)
                    # extra (new-token) column
                    for vc in range(CKV // 128):
                        nc.tensor.matmul(ps_x[:], qat[:, vc, :, b],
                                         ckvtt[:, vc, b:b + 1],
                                         start=vc == 0, stop=False)
                    for c in range(16):
                        nc.tensor.matmul(ps_x[:], qr_bd[:, c, :, b],
                                         krtt_full[:, c, b:b + 1],
                                         start=False, stop=c == 15)

                    t1 = ep.tile([NH, T_L + 1], F32, tag="t1")
                    nc.vector.scalar_tensor_tensor(
                        t1[:], msk_all[:, b, :], -1e9, colm[:],
                        op0=mybir.AluOpType.mult, op1=mybir.AluOpType.add)
                    s_all = ep.tile([NH, T_L + 1], F32, tag="s_all")
                    nc.vector.tensor_add(s_all[:, 0:T_L], ps_s[:], t1[:, 0:T_L])
                    nc.vector.tensor_add(s_all[:, T_L:T_L + 1], ps_x[:],
                                         t1[:, T_L:T_L + 1])
                    e_sb = ep.tile([NH, T_L + 1], F32, tag="e_sb")
                    den = ep.tile([NH, 1], F32, tag="den")
                    nc.scalar.activation(e_sb[:], s_all[:], AF.Exp,
                                         scale=SCALE, accum_out=den[:])

                    # transpose e -> eT [128, tc2, 32] (f32r), extra [1, 32]
                    eT = ep.tile([128, T_L // 128, NH], F32R, tag="eT")
                    exT = ep.tile([1, NH], F32R, tag="exT")
                    for tc2 in range(T_L // 128):
                        pt = pstr.tile([128, NH], F32, tag="ps_tr")
                        nc.tensor.transpose(
                            pt[:], e_sb[:, 128 * tc2:128 * (tc2 + 1)],
                            id128[0:NH, 0:NH])
                        nc.scalar.copy(eT[:, tc2, :], pt[:])
                    ptx = psx.tile([1, NH], F32, tag="ps_trx")
                    nc.tensor.transpose(ptx[:], e_sb[:, T_L:T_L + 1],
                                        id128[0:NH, 0:NH])
                    nc.scalar.copy(exT[:], ptx[:])

                    # ctxU accumulation [32, 512]
                    ps_c = psb.tile([NH, CKV], F32, tag="ps_c")
                    ckv_t = ckvp.tile([128, T_L // 128, CKV], F32R, tag="ckv")
                    nc.sync.dma_start(
                        ckv_t[:],
                        ckv_sl[b].rearrange("(tc p) v -> p tc v", p=128))
                    for tc2 in range(T_L // 128):
                        nc.tensor.matmul(ps_c[:], eT[:, tc2, :],
                                         ckv_t[:, tc2, :],
                                         start=tc2 == 0, stop=False)
                    nc.tensor.matmul(ps_c[:], exT[:], ckvt_row_r[b][:],
                                     start=False, stop=True)
                    nc.scalar.copy(rs_sb[:, b, 0:CKV], ps_c[:])
                    nc.vector.tensor_copy(rs_sb[:, b, CKV:CKV + 1], den[:])

            nc.sync.dma_start(rs_in[:], rs_sb[:])
            nc.gpsimd.collective_compute(
                "ReduceScatter", mybir.AluOpType.add, replica_groups=[CORE_IDS],
                ins=[rs_in[:]], outs=[rs_out[:]])

            # ---------------- phase C ----------------
            ctxu = pp.tile([128, CKV // 128, NH_L, B], F32)
            for vc in range(CKV // 128):
                nc.gpsimd.dma_start(
                    ctxu[:, vc, :, :],
                    rs_out[:, :, 128 * vc:128 * (vc + 1)].rearrange(
                        "n b p -> p n b"))
            den_sb = pp.tile([1, NH_L * B], F32)
            nc.gpsimd.dma_start(
                den_sb[:],
                rs_out[:, :, CKV:CKV + 1].rearrange("n b one -> one (n b)"))
            recip = pp.tile([1, NH_L * B], F32)
            nc.vector.reciprocal(recip[:], den_sb[:])
            ones_sb = pp.tile([1, 128], F32)
            nc.vector.memset(ones_sb[:], 1.0)
            recip_b = pp.tile([128, NH_L, B], F32)
            with tc.tile_pool(name="psC0", bufs=1, space="PSUM") as ps:
                pr = ps.tile([128, NH_L * B], F32)
                nc.tensor.matmul(pr[:], ones_sb[:], recip[:],
                                 start=True, stop=True)
                nc.scalar.copy(recip_b[:].rearrange("p n b -> p (n b)"), pr[:])

            wuv_sb = pp.tile([128, CKV // 128, NH_L, HD], F32)
            nc.scalar.dma_start(
                wuv_sb[:], wuv[:].rearrange("(vc p) n d -> p vc n d", p=128))
            ao_sb = pp.tile([128, NH_L, B], F32R)
            with tc.tile_pool(name="psC1", bufs=2, space="PSUM") as ps:
                for n in range(NH_L):
                    pa = ps.tile([128, B], F32, tag="ao")
                    for vc in range(CKV // 128):
                        nc.tensor.matmul(pa[:], wuv_sb[:, vc, n, :],
                                         ctxu[:, vc, n, :],
                                         start=vc == 0, stop=vc == CKV // 128 - 1)
                    ao_tmp = pp.tile([128, B], F32, name=f"ao_tmp{n}",
                                     tag=f"ao_tmp{n}")
                    nc.vector.tensor_mul(ao_tmp[:], pa[:], recip_b[:, n, :])
                    nc.scalar.copy(ao_sb[:, n, :], ao_tmp[:])

            # ---------------- phase D ----------------
            with tc.tile_pool(name="wo_s", bufs=3) as wp, \
                 tc.tile_pool(name="o_s", bufs=2) as op_, \
                 tc.tile_pool(name="psD", bufs=2, space="PSUM") as ps:
                for hc in range(H // 512):
                    po = ps.tile([B, 512], F32, tag="po")
                    w = wp.tile([128, NH_L, 512], F32R, tag="wo")
                    nc.sync.dma_start(
                        w[:], wo[:, 512 * hc:512 * (hc + 1)].rearrange(
                            "(c p) h -> p c h", p=128))
                    for c in range(NH_L):
                        nc.tensor.matmul(po[:], ao_sb[:, c, :], w[:, c, :],
                                         start=c == 0, stop=c == NH_L - 1)
                    o_sb = op_.tile([B, 512], F32, tag="o_sb")
                    nc.scalar.copy(o_sb[:], po[:])
                    nc.sync.dma_start(out_part[:, 512 * hc:512 * (hc + 1)],
                                      o_sb[:])

    _split_multi_waits(nc)
    return nc


_nc_cache = None


def _get_nc():
    global _nc_cache
    if _nc_cache is None:
        _nc_cache = build()
    return _nc_cache


def _prep_core_inputs(i, hid, mask, cached_cKV, cached_kR,
                      W_DQ, W_DKV, W_UQ_C, W_UQ_R, W_UK_C, W_UV_C, W_KR, W_O,
                      ident128, ident32=None):
    sl = slice(T_L * i, T_L * (i + 1))
    hs = slice(NH_L * i, NH_L * (i + 1))
    cm = np.zeros((NH, T_L + 1), np.float32)
    if i != NC - 1:
        cm[:, T_L] = -1e9
    return {
        "hiddenT": np.ascontiguousarray(hid.T),
        "wdq": np.ascontiguousarray(W_DQ[:, CQ_L * i:CQ_L * (i + 1)]),
        "wdkv": np.ascontiguousarray(W_DKV[:, CKV_L * i:CKV_L * (i + 1)]),
        "wuqc": np.ascontiguousarray(W_UQ_C[:, hs, :].reshape(CQ, NH_L * HD)),
        "wuqr": np.ascontiguousarray(W_UQ_R[:, hs, :].reshape(CQ, NH_L * RD)),
        "wkr": np.ascontiguousarray(W_KR[:, hs, :].reshape(H, NH_L * RD)),
        "wukt": np.ascontiguousarray(W_UK_C[:, hs, :].transpose(2, 1, 0)),
        "wuv": np.ascontiguousarray(W_UV_C[:, hs, :]),
        "wo": np.ascontiguousarray(W_O[hs].reshape(NH_L * HD, H)),
        "ckvt_sl": np.ascontiguousarray(cached_cKV[:, sl, :].transpose(0, 2, 1)),
        "ckv_sl": np.ascontiguousarray(cached_cKV[:, sl, :]),
        "krt_sl": np.ascontiguousarray(
            cached_kR[:, sl].transpose(0, 2, 3, 1).reshape(B, NH * RD, T_L)),
        "mask_sl": np.ascontiguousarray(
            np.concatenate([mask[:, 0, :, sl], mask[:, 0, :, T:T + 1]], axis=-1)),
        "colmask": cm,
        "ident128": ident128,
    }


def kernel(hidden_states, mask, cached_cKV, cached_kR,
           W_DQ, W_DKV, W_UQ_C, W_UQ_R, W_UK_C, W_UV_C, W_KR, W_O):
    hidden_states = np.asarray(hidden_states, np.float32)
    mask = np.asarray(mask, np.float32)
    cached_cKV = np.asarray(cached_cKV, np.float32)
    cached_kR = np.asarray(cached_kR, np.float32)
    args = [np.asarray(a, np.float32) for a in
            (W_DQ, W_DKV, W_UQ_C, W_UQ_R, W_UK_C, W_UV_C, W_KR, W_O)]
    hid = hidden_states[:, 0, :]
    id128 = np.eye(128, dtype=np.float32)

    in_maps = [_prep_core_inputs(i, hid, mask, cached_cKV, cached_kR,
                                 *args, id128) for i in range(NC)]
    nc = _get_nc()
    res = run_bass_kernel_spmd(nc, in_maps, CORE_IDS)
    results = res.results

    output = np.sum([results[i]["out_part"] for i in range(NC)],
                    axis=0, dtype=np.float32)[:, None, :]
    cKV_t = results[0]["ckv_t_out"]                       # (B, CKV)
    kR_t = np.concatenate(
        [results[i]["kr_t_out"].reshape(B, NH_L, RD) for i in range(NC)],
        axis=1)                                           # (B, NH, RD)
    new_cached_cKV = np.concatenate([cached_cKV, cKV_t[:, None, :]], axis=1)
    new_cached_kR = np.concatenate([cached_kR, kR_t[:, None, :, :]], axis=1)
    return output, new_cached_cKV, new_cached_kR
